# revision 3
# baseline (speedup 1.0000x reference)
"""GRU-style cell (nn_Lstmcell) on 8 Trainium2 NeuronCores.

h = (1-z)*h_prev + z*tanh((r*h_prev)@whh + x@whx + bh)
r = sigmoid([x,h_prev]@wr + br),  z = sigmoid([x,h_prev]@wz + bz)

Data-parallel over the batch dim: each of the 8 cores gets B/8 rows; the
small weight matrices are replicated.

Per-core dataflow (feature-major compute):
  - DMA x/h chunks in row-major (batch on partitions, 1KB/partition lines).
  - PE-transpose 128x128 pieces of x and h into feature-major (float32r,
    staged through PSUM, evicted to SBUF).
  - Gates r^T, z^T: float32r matmuls, weights stationary, activations
    streaming with N=512 free dim (f32r streams 1 cy/row at N>=256).
  - sigmoid/tanh + per-partition bias on ScalarE straight out of PSUM.
  - rh = r*h, blend on VectorE, all [128, 1024] feature-major ops.
  - PE-transpose h_out back to batch-major, DMA out.
"""

import numpy as np

import concourse.bass as bass
import concourse.bacc as bacc
import concourse.mybir as mybir
import concourse.tile as tile
from concourse.bass_utils import run_bass_kernel_spmd

NCORES = 8
IN = 256
H = 256
CONCAT = IN + H

F32 = mybir.dt.float32
F32R = mybir.dt.float32r
SIG = mybir.ActivationFunctionType.Sigmoid
TANH = mybir.ActivationFunctionType.Tanh

_BUILD_CACHE = {}
LAST_RESULTS = None


def _r(ap):
    return ap.bitcast(F32R)


def _build(R, reps=1):
    """Build + compile the per-core kernel for R batch rows per core."""
    CHUNK = 1024 if R % 1024 == 0 else 512
    assert R % CHUNK == 0 and CHUNK % 512 == 0
    n_chunks = R // CHUNK
    sub_per_chunk = CHUNK // 128          # 128-row subtiles per chunk
    macros_per_chunk = CHUNK // 512       # 512-row macros per chunk

    nc = bacc.Bacc(
        "TRN2", target_bir_lowering=False, debug=False, num_devices=NCORES
    )

    x_d = nc.dram_tensor("x", [R, IN], F32, kind="ExternalInput").ap()
    h_d = nc.dram_tensor("h_prev", [R, H], F32, kind="ExternalInput").ap()
    wr_d = nc.dram_tensor("wr", [CONCAT, H], F32, kind="ExternalInput").ap()
    wz_d = nc.dram_tensor("wz", [CONCAT, H], F32, kind="ExternalInput").ap()
    whh_d = nc.dram_tensor("whh", [H, H], F32, kind="ExternalInput").ap()
    whx_d = nc.dram_tensor("whx", [IN, H], F32, kind="ExternalInput").ap()
    br_d = nc.dram_tensor("br", [H], F32, kind="ExternalInput").ap()
    bz_d = nc.dram_tensor("bz", [H], F32, kind="ExternalInput").ap()
    bh_d = nc.dram_tensor("bh", [H], F32, kind="ExternalInput").ap()
    id_d = nc.dram_tensor("ident", [128, 128], F32, kind="ExternalInput").ap()
    out_d = nc.dram_tensor("h_out", [R, H], F32, kind="ExternalOutput").ap()

    x_dram = x_d.rearrange("(n p) f -> p n f", p=128)
    h_dram = h_d.rearrange("(n p) f -> p n f", p=128)
    out_dram = out_d.rearrange("(n p) f -> p n f", p=128)

    with tile.TileContext(nc) as tc:
        with (
            tc.tile_pool(name="const", bufs=1) as cpool,
            tc.tile_pool(name="io", bufs=2) as iopool,
            tc.tile_pool(name="work", bufs=2) as wpool,
            tc.tile_pool(name="psg", bufs=1, space="PSUM") as psg,
            tc.tile_pool(name="pst", bufs=2, space="PSUM") as pst,
        ):
            ident = cpool.tile([128, 128], F32R)
            nc.sync.dma_start(ident[:], _r(id_d))
            wr_sb = cpool.tile([128, 4 * H], F32R)
            nc.sync.dma_start(
                wr_sb[:].rearrange("p (c j) -> p c j", j=H),
                _r(wr_d.rearrange("(c p) j -> p c j", p=128)),
            )
            wz_sb = cpool.tile([128, 4 * H], F32R)
            nc.sync.dma_start(
                wz_sb[:].rearrange("p (c j) -> p c j", j=H),
                _r(wz_d.rearrange("(c p) j -> p c j", p=128)),
            )
            whh_sb = cpool.tile([128, 2 * H], F32R)
            nc.sync.dma_start(
                whh_sb[:].rearrange("p (c j) -> p c j", j=H),
                _r(whh_d.rearrange("(c p) j -> p c j", p=128)),
            )
            whx_sb = cpool.tile([128, 2 * H], F32R)
            nc.sync.dma_start(
                whx_sb[:].rearrange("p (c j) -> p c j", j=H),
                _r(whx_d.rearrange("(c p) j -> p c j", p=128)),
            )
            br_sb = cpool.tile([128, 2], F32)
            nc.sync.dma_start(br_sb[:], br_d.rearrange("(c p) -> p c", p=128))
            bz_sb = cpool.tile([128, 2], F32)
            nc.sync.dma_start(bz_sb[:], bz_d.rearrange("(c p) -> p c", p=128))
            bh_sb = cpool.tile([128, 2], F32)
            nc.sync.dma_start(bh_sb[:], bh_d.rearrange("(c p) -> p c", p=128))

            for ci in range(n_chunks * reps):
                ci = ci % n_chunks
                x_ch = iopool.tile([128, sub_per_chunk * IN], F32R, tag="x")
                nc.sync.dma_start(
                    x_ch[:].rearrange("p (n f) -> p n f", f=IN),
                    _r(x_dram[:, ci * sub_per_chunk : (ci + 1) * sub_per_chunk, :]),
                )
                h_ch = iopool.tile([128, sub_per_chunk * H], F32R, tag="h")
                nc.sync.dma_start(
                    h_ch[:].rearrange("p (n f) -> p n f", f=H),
                    _r(h_dram[:, ci * sub_per_chunk : (ci + 1) * sub_per_chunk, :]),
                )
                o_ch = iopool.tile([128, sub_per_chunk * H], F32, tag="o")

                for m in range(macros_per_chunk):
                    t0 = m * 4  # first 128-row subtile of this macro

                    # --- transpose x, h into feature-major [k, b] ---
                    xT = wpool.tile([128, 1024], F32R, tag="xT")
                    hT = wpool.tile([128, 1024], F32R, tag="hT")
                    for src, dstT in ((x_ch, xT), (h_ch, hT)):
                        for c in range(2):  # feature chunk
                            stg = pst.tile([128, 512], F32R, tag="stg")
                            for t in range(4):  # batch subtile
                                piece = src[
                                    :,
                                    (t0 + t) * 256 + c * 128 : (t0 + t) * 256
                                    + c * 128
                                    + 128,
                                ]
                                nc.tensor.transpose(
                                    stg[:, t * 128 : (t + 1) * 128],
                                    piece,
                                    ident[:],
                                )
                            nc.any.tensor_copy(
                                dstT[:, c * 512 : (c + 1) * 512], stg[:]
                            )

                    def xc_chunk(c):
                        # feature chunk c of [x; h]^T (c in 0..3)
                        sb = xT if c < 2 else hT
                        cc = c % 2
                        return sb[:, cc * 512 : (cc + 1) * 512]

                    # --- gates r^T, z^T: [j, b] = sum_k w[k, j] * xc^T[k, b]
                    ps_r = psg.tile([128, 1024], F32, tag="pr")
                    ps_z = psg.tile([128, 1024], F32, tag="pz")
                    for ps, w_sb in ((ps_r, wr_sb), (ps_z, wz_sb)):
                        for jc in range(2):
                            for c in range(4):
                                nc.tensor.matmul(
                                    ps[:, jc * 512 : (jc + 1) * 512],
                                    w_sb[
                                        :,
                                        c * 256 + jc * 128 : c * 256
                                        + jc * 128
                                        + 128,
                                    ],
                                    xc_chunk(c),
                                    start=(c == 0),
                                    stop=(c == 3),
                                )

                    r_sb = wpool.tile([128, 1024], F32, tag="r")
                    z_sb = wpool.tile([128, 1024], F32, tag="z")
                    for jc in range(2):
                        nc.scalar.activation(
                            r_sb[:, jc * 512 : (jc + 1) * 512],
                            ps_r[:, jc * 512 : (jc + 1) * 512],
                            SIG,
                            bias=br_sb[:, jc : jc + 1],
                        )
                        nc.scalar.activation(
                            z_sb[:, jc * 512 : (jc + 1) * 512],
                            ps_z[:, jc * 512 : (jc + 1) * 512],
                            SIG,
                            bias=bz_sb[:, jc : jc + 1],
                        )

                    # --- rh = r * h (feature-major) ---
                    rh_sb = wpool.tile([128, 1024], F32R, tag="rh")
                    nc.vector.tensor_mul(rh_sb[:], r_sb[:], hT[:].bitcast(F32))

                    # --- g^T = tanh(whh^T-part + whx-part + bh) ---
                    ps_g = psg.tile([128, 1024], F32, tag="pg")
                    for jc in range(2):
                        out_sl = ps_g[:, jc * 512 : (jc + 1) * 512]
                        for k in range(2):
                            nc.tensor.matmul(
                                out_sl,
                                whh_sb[
                                    :,
                                    k * 256 + jc * 128 : k * 256
                                    + jc * 128
                                    + 128,
                                ],
                                rh_sb[:, k * 512 : (k + 1) * 512],
                                start=(k == 0),
                                stop=False,
                            )
                        for k in range(2):
                            nc.tensor.matmul(
                                out_sl,
                                whx_sb[
                                    :,
                                    k * 256 + jc * 128 : k * 256
                                    + jc * 128
                                    + 128,
                                ],
                                xT[:, k * 512 : (k + 1) * 512],
                                start=False,
                                stop=(k == 1),
                            )

                    g_sb = wpool.tile([128, 1024], F32, tag="g")
                    for jc in range(2):
                        nc.scalar.activation(
                            g_sb[:, jc * 512 : (jc + 1) * 512],
                            ps_g[:, jc * 512 : (jc + 1) * 512],
                            TANH,
                            bias=bh_sb[:, jc : jc + 1],
                        )

                    # --- blend: ho = h + z*(g - h) (feature-major) ---
                    t1 = wpool.tile([128, 1024], F32, tag="t1")
                    nc.vector.tensor_sub(t1[:], g_sb[:], hT[:].bitcast(F32))
                    t2 = wpool.tile([128, 1024], F32, tag="t2")
                    nc.vector.tensor_mul(t2[:], z_sb[:], t1[:])
                    ho = wpool.tile([128, 1024], F32R, tag="ho")
                    nc.vector.tensor_add(ho[:], t2[:], hT[:].bitcast(F32))

                    # --- transpose h_out back to batch-major ---
                    for half in range(2):  # batch subtiles (t0+2h, t0+2h+1)
                        stg = pst.tile([128, 512], F32R, tag="stg")
                        for q in range(4):
                            t = half * 2 + q // 2  # subtile within macro
                            jc = q % 2
                            piece = ho[:, jc * 512 + t * 128 : jc * 512 + t * 128 + 128]
                            nc.tensor.transpose(
                                stg[:, q * 128 : (q + 1) * 128],
                                piece,
                                ident[:],
                            )
                        nc.any.tensor_copy(
                            o_ch[
                                :,
                                (t0 + half * 2) * 256 : (t0 + half * 2) * 256 + 512,
                            ],
                            stg[:].bitcast(F32),
                        )

                nc.sync.dma_start(
                    out_dram[:, ci * sub_per_chunk : (ci + 1) * sub_per_chunk, :],
                    o_ch[:].rearrange("p (n f) -> p n f", f=H),
                )

    nc.compile()
    return nc


def kernel(x, h_prev, wr, wz, whh, whx, br, bz, bh):
    global LAST_RESULTS
    x = np.ascontiguousarray(np.asarray(x, dtype=np.float32)).reshape(-1, IN)
    h_prev = np.ascontiguousarray(np.asarray(h_prev, dtype=np.float32)).reshape(
        -1, H
    )
    B = x.shape[0]
    assert B % NCORES == 0
    R = B // NCORES

    if R not in _BUILD_CACHE:
        _BUILD_CACHE[R] = _build(R)
    nc = _BUILD_CACHE[R]

    shared = {
        "wr": np.ascontiguousarray(np.asarray(wr, dtype=np.float32)),
        "wz": np.ascontiguousarray(np.asarray(wz, dtype=np.float32)),
        "whh": np.ascontiguousarray(np.asarray(whh, dtype=np.float32)),
        "whx": np.ascontiguousarray(np.asarray(whx, dtype=np.float32)),
        "br": np.ascontiguousarray(np.asarray(br, dtype=np.float32)),
        "bz": np.ascontiguousarray(np.asarray(bz, dtype=np.float32)),
        "bh": np.ascontiguousarray(np.asarray(bh, dtype=np.float32)),
        "ident": np.eye(128, dtype=np.float32),
    }
    in_maps = []
    for i in range(NCORES):
        m = dict(shared)
        m["x"] = x[i * R : (i + 1) * R]
        m["h_prev"] = h_prev[i * R : (i + 1) * R]
        in_maps.append(m)

    res = run_bass_kernel_spmd(nc, in_maps, list(range(NCORES)))
    LAST_RESULTS = res
    out = np.concatenate([res.results[i]["h_out"] for i in range(NCORES)], axis=0)
    return out.reshape(B, 1, H)
